# revision 8
# baseline (speedup 1.0000x reference)
"""Trainium2 Bass kernel for nn_CGLayer (gnn_message_passing).

Computation (reference semantics):
  sx[l][b,n,:]   = sum_j s_l[b,n,j,:,0]                       (j-reduction)
  cg[b,n,row,c]  = sum_i ws[b,n,(row,i)] * v9[b,n,i,c]        (CG product, 51 rows)
  mp[b,n,:]      = sum_m conn[b,n,m] * cg[b,m,:]              (message passing)
  out            = mp * 64 / ((2L+1)*||mp_L||_F)  per degree L (global norm)

Sharding over 8 cores:
  Launch 1: core (b, m-half) reduces s_l[b, m-half, :, :] over j  -> sx partials.
  Launch 2: core (b, i-half) builds cg for all m of batch b (ws precomputed on
            host from sx), then computes mp rows for its receiver half via
            TensorE matmuls (adjacency in exact bf16, cg split into bf16
            hi+lo for ~fp32 accuracy).
The final per-degree global norm couples every core's output, so it is applied
on host (3 scalars).
"""

import numpy as np
from math import factorial

import ml_dtypes

from concourse import bacc, bass, tile, mybir
from concourse.bass_utils import run_bass_kernel_spmd

B, N, C = 4, 1024, 64
HALF = N // 2
NT = N // 128          # m-tiles per batch
NCORES = 8
LOFF = [0, 1, 4]       # column offset of degree l block in the 9-wide (l,m) dim

AluOp = mybir.AluOpType
dt = mybir.dt


# ---------------------------------------------------------------- CG tables
def _cg_coeff(l1, m1, l2, m2, L, M):
    if m1 + m2 != M or not (abs(l1 - l2) <= L <= l1 + l2):
        return 0.0
    f = factorial
    pre = ((2 * L + 1) * f(L + l1 - l2) * f(L - l1 + l2) * f(l1 + l2 - L)
           / f(l1 + l2 + L + 1)) ** 0.5
    pre *= (f(L + M) * f(L - M) * f(l1 - m1) * f(l1 + m1) * f(l2 - m2)
            * f(l2 + m2)) ** 0.5
    s = 0.0
    for k in range(0, l1 + l2 - L + 1):
        dens = [k, l1 + l2 - L - k, l1 - m1 - k, l2 + m2 - k,
                L - l2 + m1 + k, L - l1 - m2 + k]
        if any(d < 0 for d in dens):
            continue
        term = (-1.0) ** k
        for d in dens:
            term /= f(d)
        s += term
    return pre * s


def _build_tables():
    rows = []
    for L in range(3):
        frags = [(l1, l2) for l1 in range(3) for l2 in range(3)
                 if abs(l1 - l2) <= L <= l1 + l2]
        for k in range(2 * L + 1):
            for (l1, l2) in frags:
                rows.append((L, k, l1, l2))
    entries = []           # per row: list of (v9col, sxcol, coeff)
    for (L, k, l1, l2) in rows:
        M = k - L
        es = []
        for i in range(2 * l1 + 1):
            m1 = i - l1
            m2 = M - m1
            if abs(m2) <= l2:
                c = _cg_coeff(l1, m1, l2, m2, L, M)
                if c != 0.0:
                    es.append((LOFF[l1] + i, LOFF[l2] + l2 + m2, c))
        entries.append(es)
    return rows, entries


ROWS, ENTRIES = _build_tables()
NROWS = len(ROWS)                                   # 51
NCH = NROWS * C                                     # 3264 output channels
# flat entry order: row-major; launch-2 ws input column e matches this order
SXCOL = np.array([sx for es in ENTRIES for (_, sx, _) in es], np.int64)
COEF = np.array([c for es in ENTRIES for (_, _, c) in es], np.float64)
NE = len(SXCOL)
# per-degree output channel ranges (rows are ordered L-major)
_L_NROWS = [sum(1 for r in ROWS if r[0] == L) for L in range(3)]
L_RANGES = []
_c0 = 0
for L in range(3):
    L_RANGES.append((_c0, _c0 + _L_NROWS[L] * C))
    _c0 += _L_NROWS[L] * C

# matmul output-channel chunks: groups of 8 rows (512 fp32 = one PSUM bank)
CHUNKS = [(r0, min(r0 + 8, NROWS)) for r0 in range(0, NROWS, 8)]

SPLIT_LO = True        # bf16 hi+lo split for the cg operand of the matmul


# ---------------------------------------------------------------- launch 1
def build_launch1():
    """Per core: s_l[b, m-half, :, :] (f32) -> sxp[m-half, 9] (sum over j)."""
    nc = bacc.Bacc("TRN2", target_bir_lowering=False, debug=False,
                   num_devices=NCORES)
    s_in = [nc.dram_tensor(f"s{l}h", [HALF, N, 2 * l + 1], dt.float32,
                           kind="ExternalInput") for l in range(3)]
    sxp_out = nc.dram_tensor("sxp", [HALF, 9], dt.float32, kind="ExternalOutput")
    with tile.TileContext(nc) as tc:
        with (tc.tile_pool(name="stream", bufs=3) as pool,
              tc.tile_pool(name="acc", bufs=1) as accpool):
            sx_sb = accpool.tile([128, HALF // 128, 9], dt.float32)
            for t in range(HALF // 128):
                for l in range(3):
                    d = 2 * l + 1
                    st = pool.tile([128, N, d], dt.float32, tag=f"s{l}")
                    nc.sync.dma_start(st[:, :, :], s_in[l][t * 128:(t + 1) * 128, :, :])
                    # reduce over j (middle axis): present AP as [p, d, j]
                    nc.vector.tensor_reduce(
                        sx_sb[:, t, LOFF[l]:LOFF[l] + d],
                        st[:, :, :].transpose([0, 2, 1]),
                        axis=mybir.AxisListType.X,
                        op=AluOp.add,
                    )
            nc.sync.dma_start(
                sxp_out.rearrange("(t p) c -> p t c", p=128), sx_sb[:, :, :])
    nc.compile()
    return nc


# ---------------------------------------------------------------- launch 2
def build_launch2():
    """Per core (b, i-half):
       v9[N,9,C] f32, ws[N,NE] f32, adjT[N,HALF] bf16 -> mp[HALF, NCH] f32."""
    nc = bacc.Bacc("TRN2", target_bir_lowering=False, debug=False,
                   num_devices=NCORES)
    v9_in = nc.dram_tensor("v9", [N, 9, C], dt.float32, kind="ExternalInput")
    ws_in = nc.dram_tensor("ws", [N, NE], dt.float32, kind="ExternalInput")
    adjT_in = nc.dram_tensor("adjT", [N, HALF], dt.bfloat16, kind="ExternalInput")
    mp_out = nc.dram_tensor("mp", [HALF, NCH], dt.float32, kind="ExternalOutput")

    e_bases = []
    _eb = 0
    for r in range(NROWS):
        e_bases.append(_eb)
        _eb += len(ENTRIES[r])

    with tile.TileContext(nc) as tc:
        with (tc.tile_pool(name="const", bufs=1) as cpool,
              tc.tile_pool(name="stream", bufs=2) as spool,
              tc.tile_pool(name="cg", bufs=2) as cgpool,
              tc.tile_pool(name="hilo", bufs=1) as hpool,
              tc.tile_pool(name="psum", bufs=1, space="PSUM") as pspool,
              tc.tile_pool(name="out", bufs=3) as outpool):
            adjT_sb = cpool.tile([128, NT, HALF], dt.bfloat16)
            for t in range(NT):
                nc.sync.dma_start(adjT_sb[:, t, :],
                                  adjT_in[t * 128:(t + 1) * 128, :])

            his, los = [], []
            for t in range(NT):
                sl = slice(t * 128, (t + 1) * 128)
                v9t = spool.tile([128, 9, C], dt.float32, tag="v9")
                nc.sync.dma_start(v9t[:, :, :], v9_in[sl, :, :])
                wst = spool.tile([128, NE], dt.float32, tag="ws")
                nc.sync.dma_start(wst[:, :], ws_in[sl, :])

                cg = cgpool.tile([128, NROWS, C], dt.float32, tag="cg")
                for r in range(NROWS):
                    es = ENTRIES[r]
                    if not es:
                        nc.vector.memset(cg[:, r, :], 0.0)
                        continue
                    for ei, (vcol, _sx, _c) in enumerate(es):
                        e = e_bases[r] + ei
                        if ei == 0:
                            nc.vector.tensor_scalar_mul(
                                cg[:, r, :], v9t[:, vcol, :], wst[:, e:e + 1])
                        else:
                            nc.vector.scalar_tensor_tensor(
                                cg[:, r, :], v9t[:, vcol, :],
                                wst[:, e:e + 1], cg[:, r, :],
                                op0=AluOp.mult, op1=AluOp.add)
                hi = hpool.tile([128, NROWS, C], dt.bfloat16, tag=f"hi{t}")
                nc.scalar.copy(hi[:, :, :], cg[:, :, :])
                his.append(hi)
                if SPLIT_LO:
                    lo = hpool.tile([128, NROWS, C], dt.bfloat16, tag=f"lo{t}")
                    nc.vector.tensor_sub(lo[:, :, :], cg[:, :, :], hi[:, :, :])
                    los.append(lo)

            npass = 2 if SPLIT_LO else 1
            for ic in range(HALF // 128):
                for ci, (r0, r1) in enumerate(CHUNKS):
                    nf = (r1 - r0) * C
                    ps = pspool.tile([128, nf], dt.float32, tag=f"ps{ci}")
                    nmm = NT * npass
                    mi = 0
                    for t in range(NT):
                        srcs = [his[t], los[t]] if SPLIT_LO else [his[t]]
                        for src in srcs:
                            nc.tensor.matmul(
                                ps[:, :],
                                adjT_sb[:, t, ic * 128:(ic + 1) * 128],
                                src[:, r0:r1, :],
                                start=(mi == 0), stop=(mi == nmm - 1))
                            mi += 1
                    ot = outpool.tile([128, nf], dt.float32, tag="out")
                    nc.scalar.copy(ot[:, :], ps[:, :])
                    nc.sync.dma_start(
                        mp_out[ic * 128:(ic + 1) * 128, r0 * C:r1 * C],
                        ot[:, :])
    nc.compile()
    return nc


_programs = {}


def _get_program(name):
    if name not in _programs:
        _programs[name] = (build_launch1 if name == "l1" else build_launch2)()
    return _programs[name]


# ---------------------------------------------------------------- host driver
def _run(nc, in_maps, **kw):
    return run_bass_kernel_spmd(nc, in_maps, list(range(NCORES)), **kw)


def kernel(v0, v1, v2, s0, s1, s2, conn, _trace=False, _results=None):
    v0 = np.asarray(v0, np.float32)
    v1 = np.asarray(v1, np.float32)
    v2 = np.asarray(v2, np.float32)
    conn = np.asarray(conn)
    s = [np.asarray(x, np.float32) for x in (s0, s1, s2)]

    core_ids = list(range(NCORES))

    # ---- launch 1: j-reduction of s, sharded (b, m-half)
    in_maps1 = []
    for k in core_ids:
        b, h = divmod(k, 2)
        msl = slice(h * HALF, (h + 1) * HALF)
        in_maps1.append({
            f"s{l}h": np.ascontiguousarray(s[l][b, msl, :, :, 0])
            for l in range(3)})
    r1 = _run(_get_program("l1"), in_maps1, trace=_trace)
    sx = np.empty((B, N, 9), np.float32)
    for k in core_ids:
        b, h = divmod(k, 2)
        sx[b, h * HALF:(h + 1) * HALF] = r1.results[k]["sxp"]

    # ---- host: ws table, v9 concat, adjacency transpose (exact in bf16)
    ws = (sx[:, :, SXCOL].astype(np.float64) * COEF).astype(np.float32)
    v9 = np.concatenate([v0, v1, v2], axis=2)                 # [B, N, 9, C]
    adjT = conn.transpose(0, 2, 1).astype(ml_dtypes.bfloat16)  # [B, m, i]

    # ---- launch 2: cg product + message-passing matmul, sharded (b, i-half)
    in_maps2 = []
    for k in core_ids:
        b, h = divmod(k, 2)
        isl = slice(h * HALF, (h + 1) * HALF)
        in_maps2.append({
            "v9": v9[b],
            "ws": ws[b],
            "adjT": np.ascontiguousarray(adjT[b, :, isl])})
    r2 = _run(_get_program("l2"), in_maps2, trace=_trace)
    mp = np.empty((B, N, NCH), np.float32)
    for k in core_ids:
        b, h = divmod(k, 2)
        mp[b, h * HALF:(h + 1) * HALF] = r2.results[k]["mp"]

    if _results is not None:
        _results.extend([r1, r2])

    # ---- host: per-degree global norm (3 scalars across all cores)
    out = np.empty_like(mp)
    for L, (c0, c1) in enumerate(L_RANGES):
        seg = mp[:, :, c0:c1]
        nf = (2 * L + 1) * np.linalg.norm(seg.astype(np.float64))
        out[:, :, c0:c1] = (seg.astype(np.float64) / (nf / C)).astype(np.float32)
    return out


# revision 13
# speedup vs baseline: 1.0560x; 1.0560x over previous
"""Trainium2 Bass kernel for nn_CGLayer (gnn_message_passing) — fused single launch.

Math (reference semantics):
  sx[b,n,g]      = sum_j s_l[b,n,j,:]                 g = (l2,m2) in [0,9)
  q[b,n,p,c]     = sx[b,n,g(p)] * v9[b,n,v(p),c]      p over 80 used products
  h[b,i,p,c]     = sum_m conn[b,i,m] * q[b,m,p,c]     (TensorE, adj exact bf16,
                                                       q split into bf16 hi+lo)
  mp[b,i,row,c]  = sum_{p in row} CG[row,p] * h[b,i,p,c]   (51 rows)
  out            = mp * 64 / ((2L+1)*||mp_L||_F)      per degree L (host, 3 scalars)

Sharding: 8 cores = (batch b, half h). Core (b,h) reduces s_l[b, :, j-half h]
(18 MiB), a pairwise AllReduce (36 KB) completes the j-sum, then the core
computes mp rows for receiver half h. The CG combine runs AFTER the
message-passing matmul (512 receiver rows instead of 1024 sender rows).
The product dimension is processed in sweeps to bound SBUF residency.
"""

import numpy as np
from math import factorial

import ml_dtypes

from concourse import bacc, tile, mybir
from concourse.bass_utils import run_bass_kernel_spmd

B, N, C = 4, 1024, 64
HALF = N // 2
NT = N // 128          # m-tiles per batch
NCORES = 8
LOFF = [0, 1, 4]

AluOp = mybir.AluOpType
dt = mybir.dt

SPLIT_LO = True        # bf16 hi+lo split of q for ~fp32 matmul accuracy


# ---------------------------------------------------------------- CG tables
def _cg_coeff(l1, m1, l2, m2, L, M):
    if m1 + m2 != M or not (abs(l1 - l2) <= L <= l1 + l2):
        return 0.0
    f = factorial
    pre = ((2 * L + 1) * f(L + l1 - l2) * f(L - l1 + l2) * f(l1 + l2 - L)
           / f(l1 + l2 + L + 1)) ** 0.5
    pre *= (f(L + M) * f(L - M) * f(l1 - m1) * f(l1 + m1) * f(l2 - m2)
            * f(l2 + m2)) ** 0.5
    s = 0.0
    for k in range(0, l1 + l2 - L + 1):
        dens = [k, l1 + l2 - L - k, l1 - m1 - k, l2 + m2 - k,
                L - l2 + m1 + k, L - l1 - m2 + k]
        if any(d < 0 for d in dens):
            continue
        term = (-1.0) ** k
        for d in dens:
            term /= f(d)
        s += term
    return pre * s


def _build_tables():
    rows = []
    for L in range(3):
        frags = [(l1, l2) for l1 in range(3) for l2 in range(3)
                 if abs(l1 - l2) <= L <= l1 + l2]
        for k in range(2 * L + 1):
            for (l1, l2) in frags:
                rows.append((L, k, l1, l2))
    entries = []           # per row: list of (v9col, sxcol, coeff)
    for (L, k, l1, l2) in rows:
        M = k - L
        es = []
        for i in range(2 * l1 + 1):
            m1 = i - l1
            m2 = M - m1
            if abs(m2) <= l2:
                c = _cg_coeff(l1, m1, l2, m2, L, M)
                if c != 0.0:
                    es.append((LOFF[l1] + i, LOFF[l2] + l2 + m2, c))
        entries.append(es)
    return rows, entries


ROWS, ENTRIES = _build_tables()
NROWS = len(ROWS)                                   # 51
NCH = NROWS * C                                     # 3264
_L_NROWS = [sum(1 for r in ROWS if r[0] == L) for L in range(3)]
L_RANGES = []
_c0 = 0
for L in range(3):
    L_RANGES.append((_c0, _c0 + _L_NROWS[L] * C))
    _c0 += _L_NROWS[L] * C

# product columns: (sxcol g)-major x (vcol), dropping unused (8,8) -> 80 cols.
GROUP_NV = [9] * 8 + [8]
NP_ = sum(GROUP_NV)                                 # 80
P_START = np.cumsum([0] + GROUP_NV).tolist()
PIDX = {}
for g in range(9):
    for v in range(GROUP_NV[g]):
        PIDX[(g, v)] = P_START[g] + v

# per-row entries as (pcol, coeff), sorted by pcol
ROW_PENTRIES = []
for es in ENTRIES:
    pes = sorted((PIDX[(sxcol, vcol)], coeff) for (vcol, sxcol, coeff) in es)
    ROW_PENTRIES.append(pes)

# sweeps over sx-groups: bounds hi/lo SBUF residency
SWEEP_GROUPS = [[0, 1, 2], [3, 4, 5], [6, 7, 8]]
SWEEPS = []   # (p0, width, [(local_chunk_off, chunk_w), ...], [(g, local_off, nv), ...])
for sg in SWEEP_GROUPS:
    p0 = P_START[sg[0]]
    width = sum(GROUP_NV[g] for g in sg)
    chunks = [(off, min(8, width - off)) for off in range(0, width, 8)]
    groups = []
    off = 0
    for g in sg:
        groups.append((g, off, GROUP_NV[g]))
        off += GROUP_NV[g]
    SWEEPS.append((p0, width, chunks, groups))
MAXW = max(w for (_, w, _, _) in SWEEPS)

# per-sweep row entries with global-first (init) marker
SWEEP_ENTRIES = []     # per sweep: list of (row, local_p, coeff, is_init)
for si, (p0, width, _, _) in enumerate(SWEEPS):
    lst = []
    for r, pes in enumerate(ROW_PENTRIES):
        for k, (p, cf) in enumerate(pes):
            if p0 <= p < p0 + width:
                lst.append((r, p - p0, cf, k == 0))
    SWEEP_ENTRIES.append(lst)


# ---------------------------------------------------------------- program
def build_fused():
    nc = bacc.Bacc("TRN2", target_bir_lowering=False, debug=False,
                   num_devices=NCORES)
    s_in = [nc.dram_tensor(f"s{l}h", [N, HALF, 2 * l + 1], dt.float32,
                           kind="ExternalInput") for l in range(3)]
    v9_in = nc.dram_tensor("v9", [N, 9, C], dt.float32, kind="ExternalInput")
    adjT_in = nc.dram_tensor("adjT", [N, HALF], dt.bfloat16, kind="ExternalInput")
    mp_out = nc.dram_tensor("mp", [HALF, NCH], dt.float32, kind="ExternalOutput")
    ar_in = nc.dram_tensor("ar_in", [N, 9], dt.float32)
    ar_out = nc.dram_tensor("ar_out", [N, 9], dt.float32)
    groups = [[0, 1], [2, 3], [4, 5], [6, 7]]
    npass = 2 if SPLIT_LO else 1

    with tile.TileContext(nc) as tc:
        with (tc.tile_pool(name="const", bufs=1) as cpool,
              tc.tile_pool(name="stream", bufs=2) as spool,
              tc.tile_pool(name="q", bufs=2) as qpool,
              tc.tile_pool(name="hilo", bufs=1) as hpool,
              tc.tile_pool(name="h4", bufs=1) as h4pool,
              tc.tile_pool(name="psum", bufs=2, space="PSUM") as pspool):
            # ---- resident tiles
            adjT_sb = cpool.tile([128, NT, HALF], dt.bfloat16)
            sxp_sb = cpool.tile([128, NT, 9], dt.float32)
            sx_sb = cpool.tile([128, NT, 9], dt.float32)
            out4 = cpool.tile([128, NT // 2, NROWS, C], dt.float32)
            for t in range(NT):
                sl = slice(t * 128, (t + 1) * 128)
                nc.sync.dma_start(adjT_sb[:, t, :], adjT_in[sl, :])

            # ---- phase A: j-half reduction of s (DVE: s0,s2; GpSimd: s1)
            for t in range(NT):
                sl = slice(t * 128, (t + 1) * 128)
                for l in (0, 2):
                    d = 2 * l + 1
                    st = spool.tile([128, HALF, d], dt.float32, tag=f"s{l}")
                    nc.sync.dma_start(st[:, :, :], s_in[l][sl, :, :])
                    nc.vector.tensor_reduce(
                        sxp_sb[:, t, LOFF[l]:LOFF[l] + d],
                        st[:, :, :].transpose([0, 2, 1]),
                        axis=mybir.AxisListType.X, op=AluOp.add)
                d = 3
                st = spool.tile([128, HALF, d], dt.float32, tag="s1")
                nc.sync.dma_start(st[:, :, :], s_in[1][sl, :, :])
                n = HALF // 2
                while n >= 1:
                    nc.gpsimd.tensor_add(
                        st[:, 0:n, :], st[:, 0:n, :], st[:, n:2 * n, :])
                    n //= 2
                nc.gpsimd.tensor_copy(sxp_sb[:, t, LOFF[1]:LOFF[1] + d],
                                      st[:, 0, :])

            # ---- phase B: pairwise AllReduce of sx partials (36 KB)
            nc.sync.dma_start(ar_in.rearrange("(t p) c -> p t c", p=128),
                              sxp_sb[:, :, :])
            nc.gpsimd.collective_compute(
                "AllReduce", AluOp.add, replica_groups=groups,
                ins=[ar_in[:]], outs=[ar_out[:]])
            nc.sync.dma_start(sx_sb[:, :, :],
                              ar_out.rearrange("(t p) c -> p t c", p=128))

            # ---- phases C/D/E per sweep
            for si, (p0, width, chunks, sgroups) in enumerate(SWEEPS):
                his, los = [], []
                for t in range(NT):
                    v9t = spool.tile([128, 9, C], dt.float32, tag="v9")
                    nc.sync.dma_start(v9t[:, :, :],
                                      v9_in[t * 128:(t + 1) * 128, :, :])
                    q = qpool.tile([128, MAXW, C], dt.float32, tag="q")
                    for (g, loff, nv) in sgroups:
                        nc.vector.tensor_scalar_mul(
                            q[:, loff:loff + nv, :], v9t[:, 0:nv, :],
                            sx_sb[:, t, g:g + 1])
                    hi = hpool.tile([128, MAXW, C], dt.bfloat16, tag=f"hi{t}")
                    nc.scalar.copy(hi[:, 0:width, :], q[:, 0:width, :])
                    his.append(hi)
                    if SPLIT_LO:
                        lo = hpool.tile([128, MAXW, C], dt.bfloat16,
                                        tag=f"lo{t}")
                        nc.gpsimd.tensor_sub(lo[:, 0:width, :],
                                             q[:, 0:width, :],
                                             hi[:, 0:width, :])
                        los.append(lo)

                h4 = h4pool.tile([128, NT // 2, MAXW, C], dt.float32, tag="h4")
                for ic in range(NT // 2):
                    for (coff, cw) in chunks:
                        ps = pspool.tile([128, cw * C], dt.float32,
                                         tag=f"ps{coff // 8 % 4}")
                        nmm = NT * npass
                        mi = 0
                        for t in range(NT):
                            srcs = [his[t], los[t]] if SPLIT_LO else [his[t]]
                            for src in srcs:
                                nc.tensor.matmul(
                                    ps[:, :],
                                    adjT_sb[:, t, ic * 128:(ic + 1) * 128],
                                    src[:, coff:coff + cw, :],
                                    start=(mi == 0), stop=(mi == nmm - 1))
                                mi += 1
                        nc.scalar.copy(
                            h4[:, ic, coff:coff + cw, :].rearrange(
                                "p a b -> p (a b)"),
                            ps[:, :])

                # ---- phase E: CG combine over all 4 receiver chunks at once
                for (r, lp, cf, is_init) in SWEEP_ENTRIES[si]:
                    if is_init:
                        nc.vector.tensor_scalar_mul(
                            out4[:, :, r, :], h4[:, :, lp, :], float(cf))
                    else:
                        nc.vector.scalar_tensor_tensor(
                            out4[:, :, r, :], h4[:, :, lp, :], float(cf),
                            out4[:, :, r, :], op0=AluOp.mult, op1=AluOp.add)

            for ic in range(NT // 2):
                nc.sync.dma_start(
                    mp_out[ic * 128:(ic + 1) * 128, :],
                    out4[:, ic, :, :].rearrange("p a b -> p (a b)"))
    nc.compile()
    return nc


_programs = {}


def _get_program():
    if "fused" not in _programs:
        _programs["fused"] = build_fused()
    return _programs["fused"]


# ---------------------------------------------------------------- host driver
def kernel(v0, v1, v2, s0, s1, s2, conn, _trace=False, _results=None):
    v0 = np.asarray(v0, np.float32)
    v1 = np.asarray(v1, np.float32)
    v2 = np.asarray(v2, np.float32)
    conn = np.asarray(conn)
    s = [np.asarray(x, np.float32) for x in (s0, s1, s2)]

    v9 = np.concatenate([v0, v1, v2], axis=2)                  # [B, N, 9, C]
    adjT = conn.transpose(0, 2, 1).astype(ml_dtypes.bfloat16)  # [B, m, i]

    core_ids = list(range(NCORES))
    in_maps = []
    for k in core_ids:
        b, h = divmod(k, 2)
        jsl = slice(h * HALF, (h + 1) * HALF)
        m = {f"s{l}h": np.ascontiguousarray(s[l][b, :, jsl, :, 0])
             for l in range(3)}
        m["v9"] = v9[b]
        m["adjT"] = np.ascontiguousarray(adjT[b, :, h * HALF:(h + 1) * HALF])
        in_maps.append(m)

    r = run_bass_kernel_spmd(_get_program(), in_maps, core_ids, trace=_trace)
    mp = np.empty((B, N, NCH), np.float32)
    for k in core_ids:
        b, h = divmod(k, 2)
        mp[b, h * HALF:(h + 1) * HALF] = r.results[k]["mp"]

    if _results is not None:
        _results.append(r)

    out = np.empty_like(mp)
    for L, (c0, c1) in enumerate(L_RANGES):
        seg = mp[:, :, c0:c1]
        nf = (2 * L + 1) * np.linalg.norm(seg.astype(np.float64))
        out[:, :, c0:c1] = (seg.astype(np.float64) / (nf / C)).astype(np.float32)
    return out


# revision 16
# speedup vs baseline: 1.3741x; 1.3013x over previous
"""Trainium2 Bass kernel for nn_CGLayer (gnn_message_passing) — fused single launch.

Math (reference semantics):
  sx[b,n,g]      = sum_j s_l[b,n,j,:]                 g = (l2,m2) in [0,9)
  q[b,n,p,c]     = sx[b,n,g(p)] * v9[b,n,v(p),c]      p over 80 used products
  h[b,i,p,c]     = sum_m conn[b,i,m] * q[b,m,p,c]     (TensorE; conn and q exact
                                                       /near-exact in fp16)
  mp[b,i,row,c]  = sum_{p in row} CG[row,p] * h[b,i,p,c]   (51 rows, fp32)
  out            = mp * 64 / ((2L+1)*||mp_L||_F)      per degree L (host, 3 scalars)

Sharding: 8 cores = (batch b, half h). Core (b,h) reduces s_l[b, :, j-half h]
(18 MiB); two pipelined pairwise AllReduces (m-tiles 0-3, then 4-7) complete
the j-sum; the core then computes mp rows for receiver half h. The CG combine
runs AFTER the message-passing matmul (512 receiver rows, 4-chunk fused).
Products are processed in column sweeps [16,32,32] so every matmul is a full
512-wide PSUM bank; matmuls run sender-tile-outer so weights amortize and
PSUM accumulation starts as soon as each sender tile's products exist.
"""

import numpy as np
from math import factorial

import ml_dtypes

from concourse import bacc, tile, mybir
from concourse.bass_utils import run_bass_kernel_spmd

B, N, C = 4, 1024, 64
HALF = N // 2
NT = N // 128          # m-tiles per batch
NIC = HALF // 128      # receiver chunks per core
NCORES = 8
LOFF = [0, 1, 4]

AluOp = mybir.AluOpType
dt = mybir.dt
QDT = dt.float16       # product dtype for the TensorE pass


# ---------------------------------------------------------------- CG tables
def _cg_coeff(l1, m1, l2, m2, L, M):
    if m1 + m2 != M or not (abs(l1 - l2) <= L <= l1 + l2):
        return 0.0
    f = factorial
    pre = ((2 * L + 1) * f(L + l1 - l2) * f(L - l1 + l2) * f(l1 + l2 - L)
           / f(l1 + l2 + L + 1)) ** 0.5
    pre *= (f(L + M) * f(L - M) * f(l1 - m1) * f(l1 + m1) * f(l2 - m2)
            * f(l2 + m2)) ** 0.5
    s = 0.0
    for k in range(0, l1 + l2 - L + 1):
        dens = [k, l1 + l2 - L - k, l1 - m1 - k, l2 + m2 - k,
                L - l2 + m1 + k, L - l1 - m2 + k]
        if any(d < 0 for d in dens):
            continue
        term = (-1.0) ** k
        for d in dens:
            term /= f(d)
        s += term
    return pre * s


def _build_tables():
    rows = []
    for L in range(3):
        frags = [(l1, l2) for l1 in range(3) for l2 in range(3)
                 if abs(l1 - l2) <= L <= l1 + l2]
        for k in range(2 * L + 1):
            for (l1, l2) in frags:
                rows.append((L, k, l1, l2))
    entries = []           # per row: list of (v9col, sxcol, coeff)
    for (L, k, l1, l2) in rows:
        M = k - L
        es = []
        for i in range(2 * l1 + 1):
            m1 = i - l1
            m2 = M - m1
            if abs(m2) <= l2:
                c = _cg_coeff(l1, m1, l2, m2, L, M)
                if c != 0.0:
                    es.append((LOFF[l1] + i, LOFF[l2] + l2 + m2, c))
        entries.append(es)
    return rows, entries


ROWS, ENTRIES = _build_tables()
NROWS = len(ROWS)                                   # 51
NCH = NROWS * C                                     # 3264
_L_NROWS = [sum(1 for r in ROWS if r[0] == L) for L in range(3)]
L_RANGES = []
_c0 = 0
for L in range(3):
    L_RANGES.append((_c0, _c0 + _L_NROWS[L] * C))
    _c0 += _L_NROWS[L] * C

# product columns: (sxcol g)-major x (vcol), dropping unused (8,8) -> 80 cols
GROUP_NV = [9] * 8 + [8]
NP_ = sum(GROUP_NV)                                 # 80
P_START = np.cumsum([0] + GROUP_NV).tolist()
PIDX = {(g, v): P_START[g] + v
        for g in range(9) for v in range(GROUP_NV[g])}

ROW_PENTRIES = []
for es in ENTRIES:
    pes = sorted((PIDX[(sxcol, vcol)], coeff) for (vcol, sxcol, coeff) in es)
    ROW_PENTRIES.append(pes)

# column-range sweeps; every sweep width is a multiple of 8 (full PSUM banks)
SWEEP_RANGES = [(0, 16), (16, 48), (48, 80)]
MAXW = max(b - a for (a, b) in SWEEP_RANGES)


def _sweep_builds(c0, c1):
    """group-write segments intersecting [c0, c1): (g, va, vb, local_off)."""
    segs = []
    for g in range(9):
        ga, gb = P_START[g], P_START[g] + GROUP_NV[g]
        a, b = max(ga, c0), min(gb, c1)
        if a < b:
            segs.append((g, a - ga, b - ga, a - c0))
    return segs


SWEEPS = []
for (c0, c1) in SWEEP_RANGES:
    w = c1 - c0
    chunks = [(off, 8) for off in range(0, w, 8)]
    entries = []
    for r, pes in enumerate(ROW_PENTRIES):
        for k, (p, cf) in enumerate(pes):
            if c0 <= p < c1:
                entries.append((r, p - c0, cf, k == 0))
    SWEEPS.append((c0, w, chunks, _sweep_builds(c0, c1), entries))


# ---------------------------------------------------------------- program
def build_fused():
    nc = bacc.Bacc("TRN2", target_bir_lowering=False, debug=False,
                   num_devices=NCORES)
    s_in = [nc.dram_tensor(f"s{l}h", [N, HALF, 2 * l + 1], dt.float32,
                           kind="ExternalInput") for l in range(3)]
    v9_in = nc.dram_tensor("v9", [N, 9, C], dt.float32, kind="ExternalInput")
    adjT_in = nc.dram_tensor("adjT", [N, HALF], QDT, kind="ExternalInput")
    mp_out = nc.dram_tensor("mp", [HALF, NCH], dt.float32, kind="ExternalOutput")
    ar_in = [nc.dram_tensor(f"ar_in{i}", [HALF, 9], dt.float32) for i in range(2)]
    ar_out = [nc.dram_tensor(f"ar_out{i}", [HALF, 9], dt.float32) for i in range(2)]
    groups = [[0, 1], [2, 3], [4, 5], [6, 7]]

    with tile.TileContext(nc) as tc:
        with (tc.tile_pool(name="const", bufs=1) as cpool,
              tc.tile_pool(name="stream", bufs=2) as spool,
              tc.tile_pool(name="hi", bufs=2) as hpool,
              tc.tile_pool(name="h4", bufs=1) as h4pool,
              tc.tile_pool(name="psum", bufs=1, space="PSUM") as pspool):
            adjT_sb = cpool.tile([128, NT, HALF], QDT)
            sxp_sb = cpool.tile([128, NT, 9], dt.float32)
            sx_sb = cpool.tile([128, NT, 9], dt.float32)
            out4 = cpool.tile([128, NIC, NROWS, C], dt.float32)
            for t in range(NT):
                nc.sync.dma_start(adjT_sb[:, t, :],
                                  adjT_in[t * 128:(t + 1) * 128, :])

            # ---- phase A: j-half reduction of s
            #   DVE: s0 all tiles, s2 tiles 0-3; GpSimd: s1 all, s2 tiles 4-7
            def reduce_dve(st, t, l, d):
                nc.vector.tensor_reduce(
                    sxp_sb[:, t, LOFF[l]:LOFF[l] + d],
                    st[:, :, :].transpose([0, 2, 1]),
                    axis=mybir.AxisListType.X, op=AluOp.add)

            def reduce_gp(st, t, l, d):
                n = HALF // 2
                while n >= 1:
                    nc.gpsimd.tensor_add(
                        st[:, 0:n, :], st[:, 0:n, :], st[:, n:2 * n, :])
                    n //= 2
                nc.gpsimd.tensor_copy(sxp_sb[:, t, LOFF[l]:LOFF[l] + d],
                                      st[:, 0, :])

            for t in range(NT):
                sl = slice(t * 128, (t + 1) * 128)
                for l in range(3):
                    d = 2 * l + 1
                    st = spool.tile([128, HALF, d], dt.float32, tag=f"s{l}")
                    nc.sync.dma_start(st[:, :, :], s_in[l][sl, :, :])
                    if l == 1 or (l == 2 and t >= NT // 2):
                        reduce_gp(st, t, l, d)
                    else:
                        reduce_dve(st, t, l, d)
                # ---- phase B: pipelined pairwise AllReduce per m-tile half
                if t == NT // 2 - 1 or t == NT - 1:
                    hf = 0 if t == NT // 2 - 1 else 1
                    tsl = slice(hf * (NT // 2), (hf + 1) * (NT // 2))
                    nc.sync.dma_start(
                        ar_in[hf].rearrange("(t p) c -> p t c", p=128),
                        sxp_sb[:, tsl, :])
                    nc.gpsimd.collective_compute(
                        "AllReduce", AluOp.add, replica_groups=groups,
                        ins=[ar_in[hf][:]], outs=[ar_out[hf][:]])
                    nc.sync.dma_start(
                        sx_sb[:, tsl, :],
                        ar_out[hf].rearrange("(t p) c -> p t c", p=128))

            # ---- phases C/D/E per sweep
            for si, (c0, w, chunks, builds, entries) in enumerate(SWEEPS):
                his = []
                for t in range(NT):
                    v9t = spool.tile([128, 9, C], dt.float32, tag="v9")
                    nc.sync.dma_start(v9t[:, :, :],
                                      v9_in[t * 128:(t + 1) * 128, :, :])
                    hi = hpool.tile([128, MAXW, C], QDT, tag=f"hi{t}")
                    for bi, (g, va, vb, loff) in enumerate(builds):
                        if bi % 3 == 2:     # 1/3 of product builds on DVE
                            nc.vector.tensor_scalar_mul(
                                hi[:, loff:loff + (vb - va), :],
                                v9t[:, va:vb, :], sx_sb[:, t, g:g + 1])
                        else:               # 2/3 on ScalarE
                            nc.scalar.activation(
                                hi[:, loff:loff + (vb - va), :],
                                v9t[:, va:vb, :],
                                mybir.ActivationFunctionType.Copy,
                                scale=sx_sb[:, t, g:g + 1])
                    his.append(hi)

                h4 = h4pool.tile([128, NIC, MAXW, C], dt.float32, tag="h4")
                # sender-tile-outer matmuls; receiver chunks in waves so live
                # PSUM banks never exceed 8
                nwave = max(1, (len(chunks) * NIC) // 8)
                icw = NIC // nwave
                for wv in range(nwave):
                    wave_ics = range(wv * icw, (wv + 1) * icw)
                    pss = {
                        (ic, ci): pspool.tile(
                            [128, cw * C], dt.float32,
                            name=f"ps_{si}_{ic}_{ci}",
                            tag=f"ps{(ic % icw) * len(chunks) + ci}")
                        for ic in wave_ics
                        for ci, (coff, cw) in enumerate(chunks)}
                    for t in range(NT):
                        for ic in wave_ics:
                            for ci, (coff, cw) in enumerate(chunks):
                                nc.tensor.matmul(
                                    pss[(ic, ci)][:, :],
                                    adjT_sb[:, t, ic * 128:(ic + 1) * 128],
                                    his[t][:, coff:coff + cw, :],
                                    start=(t == 0), stop=(t == NT - 1))
                    for ic in wave_ics:
                        for ci, (coff, cw) in enumerate(chunks):
                            nc.scalar.copy(
                                h4[:, ic, coff:coff + cw, :].rearrange(
                                    "p a b -> p (a b)"),
                                pss[(ic, ci)][:, :])

                # ---- phase E: CG combine, all receiver chunks fused
                for (r, lp, cf, is_init) in entries:
                    if is_init:
                        nc.vector.tensor_scalar_mul(
                            out4[:, :, r, :], h4[:, :, lp, :], float(cf))
                    else:
                        nc.vector.scalar_tensor_tensor(
                            out4[:, :, r, :], h4[:, :, lp, :], float(cf),
                            out4[:, :, r, :], op0=AluOp.mult, op1=AluOp.add)

            for ic in range(NIC):
                nc.sync.dma_start(
                    mp_out[ic * 128:(ic + 1) * 128, :],
                    out4[:, ic, :, :].rearrange("p a b -> p (a b)"))
    nc.compile()
    return nc


_programs = {}


def _get_program():
    if "fused" not in _programs:
        _programs["fused"] = build_fused()
    return _programs["fused"]


# ---------------------------------------------------------------- host driver
def kernel(v0, v1, v2, s0, s1, s2, conn, _trace=False, _results=None):
    v0 = np.asarray(v0, np.float32)
    v1 = np.asarray(v1, np.float32)
    v2 = np.asarray(v2, np.float32)
    conn = np.asarray(conn)
    s = [np.asarray(x, np.float32) for x in (s0, s1, s2)]

    v9 = np.concatenate([v0, v1, v2], axis=2)                  # [B, N, 9, C]
    adjT = conn.transpose(0, 2, 1).astype(np.float16)          # [B, m, i]

    core_ids = list(range(NCORES))
    in_maps = []
    for k in core_ids:
        b, h = divmod(k, 2)
        jsl = slice(h * HALF, (h + 1) * HALF)
        m = {f"s{l}h": np.ascontiguousarray(s[l][b, :, jsl, :, 0])
             for l in range(3)}
        m["v9"] = v9[b]
        m["adjT"] = np.ascontiguousarray(adjT[b, :, h * HALF:(h + 1) * HALF])
        in_maps.append(m)

    r = run_bass_kernel_spmd(_get_program(), in_maps, core_ids, trace=_trace)
    mp = np.empty((B, N, NCH), np.float32)
    for k in core_ids:
        b, h = divmod(k, 2)
        mp[b, h * HALF:(h + 1) * HALF] = r.results[k]["mp"]

    if _results is not None:
        _results.append(r)

    out = np.empty_like(mp)
    for L, (c0, c1) in enumerate(L_RANGES):
        seg = mp[:, :, c0:c1]
        nf = (2 * L + 1) * np.linalg.norm(seg.astype(np.float64))
        out[:, :, c0:c1] = (seg.astype(np.float64) / (nf / C)).astype(np.float32)
    return out


# revision 18
# speedup vs baseline: 1.6869x; 1.2276x over previous
"""Trainium2 Bass kernel for nn_CGLayer (gnn_message_passing) — fused single launch.

Math (reference semantics):
  sx[b,n,g]      = sum_j s_l[b,n,j,:]                 g = (l2,m2) in [0,9)
  q[b,n,p,c]     = sx[b,n,g(p)] * v9[b,n,v(p),c]      p over 80 used products
  h[b,i,p,c]     = sum_m conn[b,i,m] * q[b,m,p,c]     (TensorE; conn and q exact
                                                       /near-exact in fp16)
  mp[b,i,row,c]  = sum_{p in row} CG[row,p] * h[b,i,p,c]   (51 rows, fp32)
  out            = mp * 64 / ((2L+1)*||mp_L||_F)      per degree L (host, 3 scalars)

Sharding: 8 cores = (batch b, half h). Core (b,h) reduces s_l[b, :, j-half h]
(18 MiB); two pipelined pairwise AllReduces (m-tiles 0-3, then 4-7) complete
the j-sum; the core then computes mp rows for receiver half h. The CG combine
runs AFTER the message-passing matmul (512 receiver rows, 4-chunk fused).
Products are processed in column sweeps [16,32,32] so every matmul is a full
512-wide PSUM bank; matmuls run sender-tile-outer so weights amortize and
PSUM accumulation starts as soon as each sender tile's products exist.
"""

import numpy as np
from math import factorial

import ml_dtypes

from concourse import bacc, tile, mybir
from concourse.bass_utils import run_bass_kernel_spmd

B, N, C = 4, 1024, 64
HALF = N // 2
NT = N // 128          # m-tiles per batch
NIC = HALF // 128      # receiver chunks per core
NCORES = 8
LOFF = [0, 1, 4]

AluOp = mybir.AluOpType
dt = mybir.dt
QDT = dt.float16       # product dtype for the TensorE pass


# ---------------------------------------------------------------- CG tables
def _cg_coeff(l1, m1, l2, m2, L, M):
    if m1 + m2 != M or not (abs(l1 - l2) <= L <= l1 + l2):
        return 0.0
    f = factorial
    pre = ((2 * L + 1) * f(L + l1 - l2) * f(L - l1 + l2) * f(l1 + l2 - L)
           / f(l1 + l2 + L + 1)) ** 0.5
    pre *= (f(L + M) * f(L - M) * f(l1 - m1) * f(l1 + m1) * f(l2 - m2)
            * f(l2 + m2)) ** 0.5
    s = 0.0
    for k in range(0, l1 + l2 - L + 1):
        dens = [k, l1 + l2 - L - k, l1 - m1 - k, l2 + m2 - k,
                L - l2 + m1 + k, L - l1 - m2 + k]
        if any(d < 0 for d in dens):
            continue
        term = (-1.0) ** k
        for d in dens:
            term /= f(d)
        s += term
    return pre * s


def _build_tables():
    rows = []
    for L in range(3):
        frags = [(l1, l2) for l1 in range(3) for l2 in range(3)
                 if abs(l1 - l2) <= L <= l1 + l2]
        for k in range(2 * L + 1):
            for (l1, l2) in frags:
                rows.append((L, k, l1, l2))
    entries = []           # per row: list of (v9col, sxcol, coeff)
    for (L, k, l1, l2) in rows:
        M = k - L
        es = []
        for i in range(2 * l1 + 1):
            m1 = i - l1
            m2 = M - m1
            if abs(m2) <= l2:
                c = _cg_coeff(l1, m1, l2, m2, L, M)
                if c != 0.0:
                    es.append((LOFF[l1] + i, LOFF[l2] + l2 + m2, c))
        entries.append(es)
    return rows, entries


ROWS, ENTRIES = _build_tables()
NROWS = len(ROWS)                                   # 51
NCH = NROWS * C                                     # 3264
_L_NROWS = [sum(1 for r in ROWS if r[0] == L) for L in range(3)]
L_RANGES = []
_c0 = 0
for L in range(3):
    L_RANGES.append((_c0, _c0 + _L_NROWS[L] * C))
    _c0 += _L_NROWS[L] * C

# product columns: (sxcol g)-major x (vcol), dropping unused (8,8) -> 80 cols
GROUP_NV = [9] * 8 + [8]
NP_ = sum(GROUP_NV)                                 # 80
P_START = np.cumsum([0] + GROUP_NV).tolist()
PIDX = {(g, v): P_START[g] + v
        for g in range(9) for v in range(GROUP_NV[g])}

ROW_PENTRIES = []
for es in ENTRIES:
    pes = sorted((PIDX[(sxcol, vcol)], coeff) for (vcol, sxcol, coeff) in es)
    ROW_PENTRIES.append(pes)

# column-range sweeps; every sweep width is a multiple of 8 (full PSUM banks)
SWEEP_RANGES = [(0, 16), (16, 48), (48, 80)]
MAXW = max(b - a for (a, b) in SWEEP_RANGES)


def _sweep_builds(c0, c1):
    """group-write segments intersecting [c0, c1): (g, va, vb, local_off)."""
    segs = []
    for g in range(9):
        ga, gb = P_START[g], P_START[g] + GROUP_NV[g]
        a, b = max(ga, c0), min(gb, c1)
        if a < b:
            segs.append((g, a - ga, b - ga, a - c0))
    return segs


SWEEPS = []
for (c0, c1) in SWEEP_RANGES:
    w = c1 - c0
    chunks = [(off, 8) for off in range(0, w, 8)]
    entries = []
    for r, pes in enumerate(ROW_PENTRIES):
        for k, (p, cf) in enumerate(pes):
            if c0 <= p < c1:
                entries.append((r, p - c0, cf, k == 0))
    SWEEPS.append((c0, w, chunks, _sweep_builds(c0, c1), entries))


# ---------------------------------------------------------------- program
def build_fused():
    nc = bacc.Bacc("TRN2", target_bir_lowering=False, debug=False,
                   num_devices=NCORES)
    s_in = [nc.dram_tensor(f"s{l}h", [N, HALF, 2 * l + 1], dt.float32,
                           kind="ExternalInput") for l in range(3)]
    v9_in = nc.dram_tensor("v9", [N, 9, C], dt.float32, kind="ExternalInput")
    adjT_in = nc.dram_tensor("adjT", [N, HALF], QDT, kind="ExternalInput")
    mp_out = nc.dram_tensor("mp", [HALF, NCH], dt.float32, kind="ExternalOutput")
    ar_in = [nc.dram_tensor(f"ar_in{i}", [HALF, 9], dt.float32) for i in range(2)]
    ar_out = [nc.dram_tensor(f"ar_out{i}", [HALF, 9], dt.float32) for i in range(2)]
    groups = [[0, 1], [2, 3], [4, 5], [6, 7]]

    with tile.TileContext(nc) as tc:
        with (tc.tile_pool(name="const", bufs=1) as cpool,
              tc.tile_pool(name="stream", bufs=2) as spool,
              tc.tile_pool(name="hi", bufs=2) as hpool,
              tc.tile_pool(name="h4", bufs=1) as h4pool,
              tc.tile_pool(name="psum", bufs=1, space="PSUM") as pspool):
            adjT_sb = cpool.tile([128, NT, HALF], QDT)
            NTH = NT // 2
            sxp_h = [cpool.tile([128, NTH, 9], dt.float32, name=f"sxp{i}")
                     for i in range(2)]
            sx_h = [cpool.tile([128, NTH, 9], dt.float32, name=f"sx{i}")
                    for i in range(2)]
            out_p = [cpool.tile([128, 2, NROWS, C], dt.float32, name=f"out{i}")
                     for i in range(2)]
            for t in range(NT):
                nc.sync.dma_start(adjT_sb[:, t, :],
                                  adjT_in[t * 128:(t + 1) * 128, :])

            # ---- phase A: j-half reduction of s (DVE: s0,s2; GpSimd: s1)
            def reduce_dve(st, t, l, d):
                nc.vector.tensor_reduce(
                    sxp_h[t // NTH][:, t % NTH, LOFF[l]:LOFF[l] + d],
                    st[:, :, :].transpose([0, 2, 1]),
                    axis=mybir.AxisListType.X, op=AluOp.add)

            def reduce_gp(st, t, l, d):
                n = HALF // 2
                while n >= 1:
                    nc.gpsimd.tensor_add(
                        st[:, 0:n, :], st[:, 0:n, :], st[:, n:2 * n, :])
                    n //= 2
                nc.gpsimd.tensor_copy(
                    sxp_h[t // NTH][:, t % NTH, LOFF[l]:LOFF[l] + d],
                    st[:, 0, :])

            for t in range(NT):
                sl = slice(t * 128, (t + 1) * 128)
                for l in range(3):
                    d = 2 * l + 1
                    st = spool.tile([128, HALF, d], dt.float32, tag=f"s{l}")
                    nc.sync.dma_start(st[:, :, :], s_in[l][sl, :, :])
                    if l == 1:
                        reduce_gp(st, t, l, d)
                    else:
                        reduce_dve(st, t, l, d)
                # ---- phase B: pipelined pairwise AllReduce per m-tile half
                if t % NTH == NTH - 1:
                    hf = t // NTH
                    nc.sync.dma_start(
                        ar_in[hf].rearrange("(t p) c -> p t c", p=128),
                        sxp_h[hf][:, :, :])
                    nc.gpsimd.collective_compute(
                        "AllReduce", AluOp.add, replica_groups=groups,
                        ins=[ar_in[hf][:]], outs=[ar_out[hf][:]])
                    nc.sync.dma_start(
                        sx_h[hf][:, :, :],
                        ar_out[hf].rearrange("(t p) c -> p t c", p=128))

            # ---- phases C/D/E per sweep
            for si, (c0, w, chunks, builds, entries) in enumerate(SWEEPS):
                his = []
                for t in range(NT):
                    v9t = spool.tile([128, 9, C], dt.float32, tag="v9")
                    nc.sync.dma_start(v9t[:, :, :],
                                      v9_in[t * 128:(t + 1) * 128, :, :])
                    hi = hpool.tile([128, MAXW, C], QDT, tag=f"hi{t}")
                    sxt = sx_h[t // NTH]
                    for bi, (g, va, vb, loff) in enumerate(builds):
                        if bi % 3 == 2:     # 1/3 of product builds on DVE
                            nc.vector.tensor_scalar_mul(
                                hi[:, loff:loff + (vb - va), :],
                                v9t[:, va:vb, :], sxt[:, t % NTH, g:g + 1])
                        else:               # 2/3 on ScalarE
                            nc.scalar.activation(
                                hi[:, loff:loff + (vb - va), :],
                                v9t[:, va:vb, :],
                                mybir.ActivationFunctionType.Copy,
                                scale=sxt[:, t % NTH, g:g + 1])
                    his.append(hi)

                # receiver-pair h tiles; sender-tile-outer matmuls in waves so
                # live PSUM banks never exceed 8
                h4p = [h4pool.tile([128, 2, MAXW, C], dt.float32,
                                   name=f"h4_{si}_{pr}", tag=f"h4{pr}")
                       for pr in range(2)]
                nwave = max(1, (len(chunks) * NIC) // 8)
                icw = NIC // nwave
                for wv in range(nwave):
                    wave_ics = range(wv * icw, (wv + 1) * icw)
                    pss = {
                        (ic, ci): pspool.tile(
                            [128, cw * C], dt.float32,
                            name=f"ps_{si}_{ic}_{ci}",
                            tag=f"ps{(ic % icw) * len(chunks) + ci}")
                        for ic in wave_ics
                        for ci, (coff, cw) in enumerate(chunks)}
                    for t in range(NT):
                        for ic in wave_ics:
                            for ci, (coff, cw) in enumerate(chunks):
                                nc.tensor.matmul(
                                    pss[(ic, ci)][:, :],
                                    adjT_sb[:, t, ic * 128:(ic + 1) * 128],
                                    his[t][:, coff:coff + cw, :],
                                    start=(t == 0), stop=(t == NT - 1))
                    for ic in wave_ics:
                        for ci, (coff, cw) in enumerate(chunks):
                            nc.scalar.copy(
                                h4p[ic // 2][:, ic % 2, coff:coff + cw, :]
                                .rearrange("p a b -> p (a b)"),
                                pss[(ic, ci)][:, :])

                # ---- phase E: CG combine per receiver pair
                for pr in range(2):
                    for (r, lp, cf, is_init) in entries:
                        if is_init:
                            nc.vector.tensor_scalar_mul(
                                out_p[pr][:, :, r, :], h4p[pr][:, :, lp, :],
                                float(cf))
                        else:
                            nc.vector.scalar_tensor_tensor(
                                out_p[pr][:, :, r, :], h4p[pr][:, :, lp, :],
                                float(cf), out_p[pr][:, :, r, :],
                                op0=AluOp.mult, op1=AluOp.add)

            for ic in range(NIC):
                nc.sync.dma_start(
                    mp_out[ic * 128:(ic + 1) * 128, :],
                    out_p[ic // 2][:, ic % 2, :, :].rearrange(
                        "p a b -> p (a b)"))
    nc.compile()
    return nc


_programs = {}


def _get_program():
    if "fused" not in _programs:
        _programs["fused"] = build_fused()
    return _programs["fused"]


# ---------------------------------------------------------------- host driver
def kernel(v0, v1, v2, s0, s1, s2, conn, _trace=False, _results=None):
    v0 = np.asarray(v0, np.float32)
    v1 = np.asarray(v1, np.float32)
    v2 = np.asarray(v2, np.float32)
    conn = np.asarray(conn)
    s = [np.asarray(x, np.float32) for x in (s0, s1, s2)]

    v9 = np.concatenate([v0, v1, v2], axis=2)                  # [B, N, 9, C]
    adjT = conn.transpose(0, 2, 1).astype(np.float16)          # [B, m, i]

    core_ids = list(range(NCORES))
    in_maps = []
    for k in core_ids:
        b, h = divmod(k, 2)
        jsl = slice(h * HALF, (h + 1) * HALF)
        m = {f"s{l}h": np.ascontiguousarray(s[l][b, :, jsl, :, 0])
             for l in range(3)}
        m["v9"] = v9[b]
        m["adjT"] = np.ascontiguousarray(adjT[b, :, h * HALF:(h + 1) * HALF])
        in_maps.append(m)

    r = run_bass_kernel_spmd(_get_program(), in_maps, core_ids, trace=_trace)
    mp = np.empty((B, N, NCH), np.float32)
    for k in core_ids:
        b, h = divmod(k, 2)
        mp[b, h * HALF:(h + 1) * HALF] = r.results[k]["mp"]

    if _results is not None:
        _results.append(r)

    out = np.empty_like(mp)
    for L, (c0, c1) in enumerate(L_RANGES):
        seg = mp[:, :, c0:c1]
        nf = (2 * L + 1) * np.linalg.norm(seg.astype(np.float64))
        out[:, :, c0:c1] = (seg.astype(np.float64) / (nf / C)).astype(np.float32)
    return out
